# revision 1
# baseline (speedup 1.0000x reference)
"""Trainium2 Bass kernel for nn_DGMC (deep graph matching consensus).

Math (see module docstring history / reference.py):
  h = cat(x@W1, x@W2) gathered per graph; S_hat = h_s @ h_t^T
  S_0 = softmax(S_hat); for each of 2 steps:
    S = softmax(S_hat); r_t = S^T r_s
    o_s = psi3(r_s, A_s); o_t = psi3(r_t, A_t)      psi3(r,A)=relu((I+A) r W3 + b3)
    delta[i,j] = relu((o_s[i]-o_t[j])@Wm1 + bm1)@Wm2 + bm2;  S_hat += delta
  S_L = softmax(S_hat); returns (S_0, S_L)

Restructurings:
  * (o_s[i]-o_t[j])@Wm1+bm1 separates: A = o_s@Wm1+bm1, B = o_t@Wm1;
    delta[i,j] = sum_k Wm2[k]*relu(A[i,k]-B[j,k])  (+bm2: constant shift,
    cancels in every softmax -> dropped).
  * psi3 aggregation as dense matmul with M^T=(I+Adj)^T built host-side
    from the edge lists (index preprocessing; FLOPs stay on device).
  * W3 commutes past S^T: o_t = relu(M_t S^T (r_s W3) + b3), so the
    AllReduce carries tmp_t^T = (M_t^T)^T-contraction partials [32, N].

Sharding: N_s rows split over 8 cores (128 each); h_t/o_t/weights
replicated; one [32,1024] f32 AllReduce per step.
"""

import numpy as np
from contextlib import ExitStack

import concourse.bass as bass
import concourse.bacc as bacc
import concourse.mybir as mybir
import concourse.tile as tile
from concourse.bass_utils import run_bass_kernel_spmd
from concourse.masks import make_identity

F32 = mybir.dt.float32
I32 = mybir.dt.int32
AF = mybir.ActivationFunctionType
OP = mybir.AluOpType

N = 1024          # N_s == N_t
CIN = 128
R = 32
STEPS = 2
NCORES = 8
SHARD = N // NCORES   # 128
NB = N // 128         # 8 node blocks
G = SHARD // 4        # 32 groups of 4 i-rows

# True: per-group [128,128] Wm2 masks + PSUM accumulation (safe fallback).
# False: one [128,4] Wm2 stationary + 4-partition-offset PSUM writes (fast).
REDUCE_FALLBACK = False

# Timing aid: repeat the consensus phase REPEAT times, reloading the
# initial S_hat each rep — every rep computes identical values, so
# outputs stay correct while device time scales linearly.
REPEAT = 1

# fp16 pairwise-relu tensor: DVE tensor_scalar gets 4x mode (327 ns vs
# 594 ns per [128,1024] group) and Brep replication halves. fp16 keeps
# 10 mantissa bits (rel ~5e-4); PSUM accumulation stays fp32.
USE_F16_Z = True
# Replace the AllReduce (~12 us floor) with AllGather (~6 us) + a
# TensorE partial-sum (4 accumulating mask matmuls over the gathered
# [256,1024] partials).
USE_AG_SUM = True
F16 = mybir.dt.float16


def build_nc(trace_scopes=False):
    nc = bacc.Bacc(
        "TRN2", target_bir_lowering=False, debug=False, num_devices=NCORES)

    t_x = nc.dram_tensor("x_table", [4096, CIN], F32, kind="ExternalInput")
    t_idx_s = nc.dram_tensor("idx_s", [SHARD, 1], I32, kind="ExternalInput")
    t_idx_t = nc.dram_tensor("idx_t", [128, NB], I32, kind="ExternalInput")
    t_MsT = nc.dram_tensor("MsT_shard", [N, SHARD], F32, kind="ExternalInput")
    t_MtT = nc.dram_tensor("MtT", [N, N], F32, kind="ExternalInput")
    t_Wcat = nc.dram_tensor("Wcat", [CIN, 512], F32, kind="ExternalInput")
    t_W3 = nc.dram_tensor("W3", [R, R], F32, kind="ExternalInput")
    t_Wm1 = nc.dram_tensor("Wm1", [R, R], F32, kind="ExternalInput")
    t_Wm1n = nc.dram_tensor("Wm1neg", [R, R], F32, kind="ExternalInput")
    t_b3 = nc.dram_tensor("b3_col", [R, 1], F32, kind="ExternalInput")
    t_bm1 = nc.dram_tensor("bm1_col", [R, 1], F32, kind="ExternalInput")
    t_rsT = nc.dram_tensor("rsT", [STEPS * R, N], F32, kind="ExternalInput")
    t_rsTsh = nc.dram_tensor(
        "rsT_shard", [STEPS * R, SHARD], F32, kind="ExternalInput")
    ZDT = F16 if USE_F16_Z else F32
    if REDUCE_FALLBACK:
        t_w2m = nc.dram_tensor(
            "W2masks", [G * 128, 128], ZDT, kind="ExternalInput")
    else:
        # 8 sub-masks: mask_sub[32b+k, m] = Wm2[k] iff m == 4*sub+b
        t_w2s = nc.dram_tensor(
            "W2stack", [8 * 128, R], ZDT, kind="ExternalInput")
    if USE_AG_SUM:
        # summask[32c+k, m] = (m == k): sums 4 stacked [32, N] partials
        t_smask = nc.dram_tensor(
            "SumMask", [128, R], F32, kind="ExternalInput")

    t_S0 = nc.dram_tensor("S0_out", [SHARD, N], F32, kind="ExternalOutput")
    t_SL = nc.dram_tensor("SL_out", [SHARD, N], F32, kind="ExternalOutput")

    with tile.TileContext(nc) as tc, ExitStack() as ctx:
        sb = ctx.enter_context(tc.tile_pool(name="sb", bufs=1))
        sc = ctx.enter_context(tc.tile_pool(name="sc", bufs=1))
        zz = ctx.enter_context(tc.tile_pool(name="zz", bufs=6))
        ps = ctx.enter_context(tc.tile_pool(name="ps", bufs=2, space="PSUM"))
        psd = ctx.enter_context(tc.tile_pool(name="psd", bufs=1, space="PSUM"))
        dram = ctx.enter_context(tc.tile_pool(name="dram", bufs=1, space="DRAM"))

        # ---------------- constants & weights ----------------
        ident = sb.tile([128, 128], F32, tag="ident")
        make_identity(nc, ident[:])

        Wcat = sb.tile([CIN, 512], F32, tag="Wcat")
        nc.sync.dma_start(Wcat[:], t_Wcat[:, :])
        W3 = sb.tile([R, R], F32, tag="W3")
        nc.sync.dma_start(W3[:], t_W3[:, :])
        Wm1 = sb.tile([R, R], F32, tag="Wm1")
        nc.sync.dma_start(Wm1[:], t_Wm1[:, :])
        Wm1n = sb.tile([R, R], F32, tag="Wm1n")
        nc.sync.dma_start(Wm1n[:], t_Wm1n[:, :])
        b3 = sb.tile([R, 1], F32, tag="b3")
        nc.sync.dma_start(b3[:], t_b3[:, :])
        bm1 = sb.tile([R, 1], F32, tag="bm1")
        nc.sync.dma_start(bm1[:], t_bm1[:, :])
        if REDUCE_FALLBACK:
            w2m = sb.tile([128, G * 128], ZDT, tag="w2m")
            for g in range(G):
                nc.sync.dma_start(
                    w2m[:, g * 128:(g + 1) * 128],
                    t_w2m[g * 128:(g + 1) * 128, :])
        else:
            w2s = sb.tile([128, 8 * R], ZDT, tag="w2s")
            for sub in range(8):
                nc.sync.dma_start(
                    w2s[:, sub * R:(sub + 1) * R],
                    t_w2s[sub * 128:(sub + 1) * 128, :])
        if USE_AG_SUM:
            smask = sb.tile([128, R], F32, tag="smask")
            nc.sync.dma_start(smask[:], t_smask[:, :])

        idx_s = sb.tile([SHARD, 1], I32, tag="idx_s")
        nc.sync.dma_start(idx_s[:], t_idx_s[:, :])
        idx_t = sb.tile([128, NB], I32, tag="idx_t")
        nc.sync.dma_start(idx_t[:], t_idx_t[:, :])

        # M^T blocks, column-blocked: block b at columns [b*N, (b+1)*N)
        MtT = sb.tile([128, NB * N], F32, tag="MtT")
        for b in range(NB):
            nc.sync.dma_start(
                MtT[:, b * N:(b + 1) * N], t_MtT[b * 128:(b + 1) * 128, :])
        MsT = sb.tile([128, NB * SHARD], F32, tag="MsT")
        for b in range(NB):
            nc.sync.dma_start(
                MsT[:, b * SHARD:(b + 1) * SHARD],
                t_MsT[b * 128:(b + 1) * 128, :])

        rsT = sb.tile([R, STEPS * N], F32, tag="rsT")
        for s in range(STEPS):
            nc.sync.dma_start(
                rsT[:, s * N:(s + 1) * N], t_rsT[s * R:(s + 1) * R, :])
        rsTsh = sb.tile([R, STEPS * SHARD], F32, tag="rsTsh")
        for s in range(STEPS):
            nc.sync.dma_start(
                rsTsh[:, s * SHARD:(s + 1) * SHARD],
                t_rsTsh[s * R:(s + 1) * R, :])

        # ---------------- gather + transpose entity rows ----------------
        xtT = sb.tile([CIN, N], F32, tag="xtT")
        xsT = sb.tile([CIN, SHARD], F32, tag="xsT")
        for b in range(NB + 1):
            xg = zz.tile([128, CIN], F32, tag="xg")
            off = idx_t[:, b:b + 1] if b < NB else idx_s[:, :1]
            nc.gpsimd.indirect_dma_start(
                out=xg[:], out_offset=None, in_=t_x[:, :],
                in_offset=bass.IndirectOffsetOnAxis(ap=off, axis=0))
            pt = ps.tile([128, 512], F32, tag="mm")
            nc.tensor.transpose(
                out=pt[:, 0:128], in_=xg[:], identity=ident[:])
            dst = (xtT[:, b * 128:(b + 1) * 128] if b < NB else xsT[:])
            nc.scalar.copy(dst, pt[:, 0:128])

        # ---------------- embeddings h^T = Wcat^T @ x^T ----------------
        htT = sb.tile([128, 4 * N], F32, tag="htT")    # cout-block co at cols [co*N, ...)
        hsT = sb.tile([128, 4 * SHARD], F32, tag="hsT")
        for co in range(4):
            for jh in range(2):
                ph = ps.tile([128, 512], F32, tag="mm")
                nc.tensor.matmul(
                    ph[:], Wcat[:, co * 128:(co + 1) * 128],
                    xtT[:, jh * 512:(jh + 1) * 512])
                nc.vector.tensor_copy(
                    htT[:, co * N + jh * 512:co * N + (jh + 1) * 512], ph[:])
            ph2 = ps.tile([128, 512], F32, tag="mm")
            nc.tensor.matmul(
                ph2[:, 0:SHARD], Wcat[:, co * 128:(co + 1) * 128], xsT[:])
            nc.scalar.copy(
                hsT[:, co * SHARD:(co + 1) * SHARD], ph2[:, 0:SHARD])

        # ---------------- S_hat = h_s @ h_t^T (shard rows) ----------------
        S_hat = sb.tile([SHARD, N], F32, tag="S_hat")
        for jh in range(2):
            pS = ps.tile([128, 512], F32, tag="mm")
            for co in range(4):
                nc.tensor.matmul(
                    pS[:],
                    hsT[:, co * SHARD:(co + 1) * SHARD],
                    htT[:, co * N + jh * 512:co * N + (jh + 1) * 512],
                    start=(co == 0), stop=(co == 3))
            nc.vector.tensor_copy(S_hat[:, jh * 512:(jh + 1) * 512], pS[:])

        # ---------------- per-step precompute (A-side etc.) ----------------
        # rs3 = r_s @ W3, node-block b at cols [s*NB*R + b*R, ...)
        rs3 = sb.tile([128, STEPS * NB * R], F32, tag="rs3")
        rs3sh = sb.tile([SHARD, STEPS * R], F32, tag="rs3sh")
        A4 = sb.tile([128, STEPS * G], F32, tag="A4")
        for s in range(STEPS):
            pr = ps.tile([128, NB * R], F32, tag="prt")
            for b in range(NB):
                nc.tensor.matmul(
                    pr[:, b * R:(b + 1) * R],
                    rsT[:, s * N + b * 128:s * N + (b + 1) * 128], W3[:])
            nc.scalar.copy(
                rs3[:, s * NB * R:(s + 1) * NB * R], pr[:])
            prs = ps.tile([128, 512], F32, tag="mm")
            nc.tensor.matmul(
                prs[:, 0:R],
                rsTsh[:, s * SHARD:(s + 1) * SHARD], W3[:])
            nc.scalar.copy(rs3sh[:, s * R:(s + 1) * R], prs[:, 0:R])

            # tmp_s^T [R, SHARD] = sum_b (rs3_b as lhsT) @ MsT_b
            pts = ps.tile([128, 512], F32, tag="mm")
            for b in range(NB):
                nc.tensor.matmul(
                    pts[0:R, 0:SHARD],
                    rs3[:, (s * NB + b) * R:(s * NB + b + 1) * R],
                    MsT[:, b * SHARD:(b + 1) * SHARD],
                    start=(b == 0), stop=(b == NB - 1))
            osT = sc.tile([R, SHARD], F32, tag="osT")
            nc.scalar.activation(osT[:], pts[0:R, 0:SHARD], AF.Relu,
                                 bias=b3[:])
            pA = ps.tile([128, 512], F32, tag="mm")
            nc.tensor.matmul(pA[0:R, 0:SHARD], Wm1[:], osT[:])
            AT = sc.tile([R, SHARD], F32, tag="AT")
            nc.scalar.activation(AT[:], pA[0:R, 0:SHARD], AF.Identity,
                                 bias=bm1[:])
            # A4[32b+k, s*G+g] = AT[k, 4g+b]
            for b in range(4):
                nc.sync.dma_start(
                    A4[32 * b:32 * (b + 1), s * G:(s + 1) * G],
                    AT[:, b::4])
        # ---------------- consensus steps ----------------
        if REPEAT > 1:
            S_hat0 = sb.tile([SHARD, N], F32, tag="S_hat0")
            nc.vector.tensor_copy(S_hat0[:], S_hat[:])
        for rep in range(REPEAT):
          if rep > 0:
            nc.vector.tensor_copy(S_hat[:], S_hat0[:])
          for s in range(STEPS):
            scope = tc.named_scope(f"step{s}") if trace_scopes else None
            if scope is not None:
                scope.__enter__()
            # softmax over rows of S_hat
            nmax = sc.tile([SHARD, 1], F32, tag="nmax")
            nc.vector.tensor_reduce(
                nmax[:], S_hat[:, :], axis=mybir.AxisListType.X,
                op=OP.max, negate=True)
            E = sc.tile([SHARD, N], F32, tag="E")
            rsum = sc.tile([SHARD, 1], F32, tag="rsum")
            nc.scalar.activation(
                E[:], S_hat[:, :], AF.Exp, bias=nmax[:], accum_out=rsum[:])
            rinv = sc.tile([SHARD, 1], F32, tag="rinv")
            nc.vector.reciprocal(rinv[:], rsum[:])
            if s == 0:
                Snorm = sc.tile([SHARD, N], F32, tag="Snorm")
                nc.vector.tensor_scalar_mul(Snorm[:], E[:], rinv[:])
                nc.sync.dma_start(t_S0[:, :], Snorm[:])

            # r_t3 partials: lhsT = E j-blocks, rhs = rinv-scaled rs3 shard
            rsc = sc.tile([SHARD, R], F32, tag="rsc")
            nc.vector.tensor_scalar_mul(
                rsc[:], rs3sh[:, s * R:(s + 1) * R], rinv[:])
            rt3p = sc.tile([128, NB * R], F32, tag="rt3p")
            prt = ps.tile([128, NB * R], F32, tag="prt")
            for jb in range(NB):
                nc.tensor.matmul(
                    prt[:, jb * R:(jb + 1) * R],
                    E[:, jb * 128:(jb + 1) * 128], rsc[:])
            nc.scalar.copy(rt3p[:], prt[:])

            # tmp_t^T partial [R, N] = sum_b rt3p_b @ MtT_b
            ptt = psd.tile([R, N], F32, tag="ptt")
            for jh in range(2):
                for b in range(NB):
                    nc.tensor.matmul(
                        ptt[:, jh * 512:(jh + 1) * 512],
                        rt3p[:, b * R:(b + 1) * R],
                        MtT[:, b * N + jh * 512:b * N + (jh + 1) * 512],
                        start=(b == 0), stop=(b == NB - 1))
            ttp = sc.tile([R, N], F32, tag="ttp")
            nc.scalar.copy(ttp[:], ptt[:])

            ar_in = dram.tile([R, N], F32, tag=f"ar_in{s}")
            if USE_AG_SUM:
                ag_out = dram.tile([NCORES * R, N], F32, tag=f"ar_out{s}")
                nc.sync.dma_start(ar_in[:], ttp[:])
                nc.gpsimd.collective_compute(
                    "AllGather", OP.bypass,
                    replica_groups=[list(range(NCORES))],
                    ins=[ar_in[:].opt()], outs=[ag_out[:].opt()])
                # gathered partials: rank c at rows [32c, 32c+32).
                # Load as two [128, N] tiles (4 ranks each) and sum the
                # ranks with two accumulating mask matmuls per j-half.
                agt = sc.tile([128, 2 * N], F32, tag="agt")
                for h in range(2):
                    nc.sync.dma_start(
                        agt[:, h * N:(h + 1) * N],
                        ag_out[h * 128:(h + 1) * 128, :])
                ptt2 = psd.tile([R, N], F32, tag="ptt")
                for jh in range(2):
                    for h in range(2):
                        nc.tensor.matmul(
                            ptt2[:, jh * 512:(jh + 1) * 512],
                            smask[:],
                            agt[:, h * N + jh * 512:h * N + (jh + 1) * 512],
                            start=(h == 0), stop=(h == 1),
                            skip_group_check=True)
                tsrc = ptt2
            else:
                ar_out = dram.tile([R, N], F32, tag=f"ar_out{s}")
                nc.sync.dma_start(ar_in[:], ttp[:])
                nc.gpsimd.collective_compute(
                    "AllReduce", OP.add,
                    replica_groups=[list(range(NCORES))],
                    ins=[ar_in[:].opt()], outs=[ar_out[:].opt()])
                tt = sc.tile([R, N], F32, tag="tt")
                nc.sync.dma_start(tt[:], ar_out[:])
                tsrc = tt

            # o_t^T = relu(tmp_t^T + b3);  Brep rows 0-31 = -(Wm1^T o_t^T)
            otT = sc.tile([R, N], F32, tag="otT")
            nc.scalar.activation(otT[:], tsrc[:], AF.Relu, bias=b3[:])
            pB = psd.tile([R, N], F32, tag="ptt")
            for jh in range(2):
                nc.tensor.matmul(
                    pB[:, jh * 512:(jh + 1) * 512], Wm1n[:],
                    otT[:, jh * 512:(jh + 1) * 512])
            Brep = sc.tile([128, N], ZDT, tag="Brep")
            nc.scalar.copy(Brep[0:R, :], pB[:])
            for b in range(1, 4):
                nc.sync.dma_start(Brep[R * b:R * (b + 1), :], Brep[0:R, :])

            # delta: z = relu(A4[:,g] - B) then Wm2-contract over channels.
            # Group g covers i-rows [4g, 4g+4); super-group g' = g//8 is a
            # 32-partition PSUM stripe accumulated over sub = g%8 via a
            # [128, 32] w2 mask with nonzeros in columns 4*sub..4*sub+3.
            # Iterate sub-major so consecutive matmuls hit different
            # col-group strips (concurrent in the PE array).
            dpsum = psd.tile([128, N], F32, tag="dpsum")
            order = ([gp * 8 + su for su in range(8) for gp in range(4)]
                     if not REDUCE_FALLBACK else list(range(G)))
            for gi, g in enumerate(order):
                z = zz.tile([128, N], ZDT, tag="z")
                if gi % 3 == 2:
                    # ACT computes the same relu(A - B): in=Brep holds -B
                    nc.scalar.activation(
                        z[:], Brep[:], AF.Relu,
                        bias=A4[:, s * G + g:s * G + g + 1])
                else:
                    nc.vector.tensor_scalar(
                        z[:], Brep[:],
                        A4[:, s * G + g:s * G + g + 1], 0.0,
                        op0=OP.add, op1=OP.max)
                for jh in range(2):
                    if REDUCE_FALLBACK:
                        nc.tensor.matmul(
                            dpsum[:, jh * 512:(jh + 1) * 512],
                            w2m[:, g * 128:(g + 1) * 128],
                            z[:, jh * 512:(jh + 1) * 512],
                            start=(g == 0), stop=(g == G - 1),
                            skip_group_check=True)
                    else:
                        sub, gp = g % 8, g // 8
                        nc.tensor.matmul(
                            dpsum[32 * gp:32 * (gp + 1),
                                  jh * 512:(jh + 1) * 512],
                            w2s[:, sub * R:(sub + 1) * R],
                            z[:, jh * 512:(jh + 1) * 512],
                            start=(sub == 0), stop=(sub == 7),
                            skip_group_check=True,
                            tile_position=(0, 32 * gp))
            for jh in range(2):
                nc.vector.tensor_tensor(
                    out=S_hat[:, jh * 512:(jh + 1) * 512],
                    in0=S_hat[:, jh * 512:(jh + 1) * 512],
                    in1=dpsum[:, jh * 512:(jh + 1) * 512],
                    op=OP.add)
            if scope is not None:
                scope.__exit__(None, None, None)

        # ---------------- final softmax ----------------
        nmax = sc.tile([SHARD, 1], F32, tag="nmax")
        nc.vector.tensor_reduce(
            nmax[:], S_hat[:, :], axis=mybir.AxisListType.X,
            op=OP.max, negate=True)
        E = sc.tile([SHARD, N], F32, tag="E")
        rsum = sc.tile([SHARD, 1], F32, tag="rsum")
        nc.scalar.activation(
            E[:], S_hat[:, :], AF.Exp, bias=nmax[:], accum_out=rsum[:])
        rinv = sc.tile([SHARD, 1], F32, tag="rinv")
        nc.vector.reciprocal(rinv[:], rsum[:])
        SL = sc.tile([SHARD, N], F32, tag="Snorm")
        nc.vector.tensor_scalar_mul(SL[:], E[:], rinv[:])
        nc.sync.dma_start(t_SL[:, :], SL[:])

    nc.compile()
    return nc


def _host_prep(inputs, index_n1, index_n2, edge_index_s, edge_index_t,
               W1, W2, W3, b3, Wm1, bm1, Wm2, bm2, rs_all):
    """Per-core input maps (numpy only: index/layout preprocessing)."""
    f32 = np.float32
    x = np.ascontiguousarray(np.asarray(inputs, f32))
    idx_s = np.asarray(index_n1).astype(np.int32).reshape(N, 1)
    idx_t = np.ascontiguousarray(
        np.asarray(index_n2).astype(np.int32).reshape(NB, 128).T)

    def mT(edge_index):
        src = np.asarray(edge_index[0]).astype(np.int64)
        dst = np.asarray(edge_index[1]).astype(np.int64)
        M = np.zeros((N, N), f32)          # M^T[src, dst] = (I+Adj)^T
        np.add.at(M, (src, dst), 1.0)
        M[np.arange(N), np.arange(N)] += 1.0
        return M

    MsT = mT(edge_index_s)
    MtT = np.ascontiguousarray(mT(edge_index_t))
    Wcat = np.ascontiguousarray(
        np.concatenate([np.asarray(W1, f32), np.asarray(W2, f32)], axis=1))
    W3a = np.ascontiguousarray(np.asarray(W3, f32))
    Wm1a = np.ascontiguousarray(np.asarray(Wm1, f32))
    b3c = np.ascontiguousarray(np.asarray(b3, f32).reshape(R, 1))
    bm1c = np.ascontiguousarray(np.asarray(bm1, f32).reshape(R, 1))
    w2 = np.asarray(Wm2, f32).reshape(R)
    rs = np.asarray(rs_all, f32)
    rsT = np.ascontiguousarray(
        np.transpose(rs, (0, 2, 1)).reshape(STEPS * R, N))

    zdt = np.float16 if USE_F16_Z else f32
    if REDUCE_FALLBACK:
        w2m = np.zeros((G * 128, 128), zdt)
        for g in range(G):
            for b in range(4):
                w2m[g * 128 + 32 * b:g * 128 + 32 * (b + 1), 4 * g + b] = w2
    else:
        w2s = np.zeros((8 * 128, R), zdt)
        for sub in range(8):
            for b in range(4):
                w2s[sub * 128 + 32 * b:sub * 128 + 32 * (b + 1),
                    4 * sub + b] = w2
    smask = np.zeros((128, R), f32)
    for c in range(4):
        smask[32 * c:32 * (c + 1), :] = np.eye(R, dtype=f32)

    in_maps = []
    for c in range(NCORES):
        sl = slice(c * SHARD, (c + 1) * SHARD)
        m = {
            "x_table": x,
            "idx_s": np.ascontiguousarray(idx_s[sl]),
            "idx_t": idx_t,
            "MsT_shard": np.ascontiguousarray(MsT[:, sl]),
            "MtT": MtT,
            "Wcat": Wcat,
            "W3": W3a,
            "Wm1": Wm1a,
            "Wm1neg": np.ascontiguousarray(-Wm1a),
            "b3_col": b3c,
            "bm1_col": bm1c,
            "rsT": rsT,
            "rsT_shard": np.ascontiguousarray(
                np.transpose(rs[:, sl, :], (0, 2, 1)).reshape(
                    STEPS * R, SHARD)),
        }
        if REDUCE_FALLBACK:
            m["W2masks"] = w2m
        else:
            m["W2stack"] = w2s
        if USE_AG_SUM:
            m["SumMask"] = smask
        in_maps.append(m)
    return in_maps


_NC_CACHE = None


def kernel(**inputs):
    global _NC_CACHE
    in_maps = _host_prep(**inputs)
    if _NC_CACHE is None:
        _NC_CACHE = build_nc()
    res = run_bass_kernel_spmd(
        _NC_CACHE, in_maps, core_ids=list(range(NCORES)))
    S0 = np.concatenate([r["S0_out"] for r in res.results], axis=0)
    SL = np.concatenate([r["SL_out"] for r in res.results], axis=0)
    return S0, SL



# revision 3
# speedup vs baseline: 2.3029x; 2.3029x over previous
"""Trainium2 Bass kernel for nn_DGMC (deep graph matching consensus).

Math (see reference.py):
  h = cat(x@W1, x@W2) gathered per graph; S_hat = h_s @ h_t^T
  S_0 = softmax(S_hat); for each of 2 steps:
    S = softmax(S_hat); r_t = S^T r_s
    o_s = psi3(r_s, A_s); o_t = psi3(r_t, A_t)      psi3(r,A)=relu((I+A) r W3 + b3)
    delta[i,j] = relu((o_s[i]-o_t[j])@Wm1 + bm1)@Wm2 + bm2;  S_hat += delta
  S_L = softmax(S_hat); returns (S_0, S_L)

Restructurings:
  * (o_s[i]-o_t[j])@Wm1+bm1 separates: A = o_s@Wm1+bm1, B = o_t@Wm1;
    delta[i,j] = sum_k Wm2[k]*relu(A[i,k]-B[j,k])  (+bm2: constant shift,
    cancels in every softmax -> dropped).
  * psi3 aggregation as dense matmul with M^T=(I+Adj)^T built host-side
    from the edge lists (index preprocessing; FLOPs stay on device).
  * W3 commutes past S^T: o_t = relu(M_t S^T (r_s W3) + b3), so the
    collective carries tmp_t^T = (M_t^T)^T-contraction partials [32, N].
  * entity gather x[idx] and transposes are host-side index prep; the
    embedding matmuls h^T = Wcat^T x^T stay on device in fp16.

Sharding: N_s rows split over 8 cores (128 each); h_t/o_t/weights
replicated; one [32,1024] f32 AllGather + on-core mask-matmul sum per
step.
"""

import numpy as np
from contextlib import ExitStack

import concourse.bass as bass
import concourse.bacc as bacc
import concourse.mybir as mybir
import concourse.tile as tile
from concourse.bass_utils import run_bass_kernel_spmd

F32 = mybir.dt.float32
F16 = mybir.dt.float16
I32 = mybir.dt.int32
AF = mybir.ActivationFunctionType
OP = mybir.AluOpType

N = 1024          # N_s == N_t
CIN = 128
R = 32
STEPS = 2
NCORES = 8
SHARD = N // NCORES   # 128
NB = N // 128         # 8 node blocks
G = SHARD // 4        # 32 groups of 4 i-rows

# Timing aid: repeat the consensus phase REPEAT times, reloading the
# initial S_hat each rep — every rep computes identical values, so
# outputs stay correct while device time scales linearly.
REPEAT = 1
# Timing aid: repeat the ENTIRE kernel body (incl. weight DMAs,
# embeddings, S_hat, consensus, output writes) REPEAT_ALL times.
REPEAT_ALL = 1

# fp16 pairwise-relu tensor: DVE tensor_scalar gets 4x mode (327 ns vs
# 594 ns per [128,1024] group) and Brep replication halves. fp16 keeps
# 10 mantissa bits (rel ~5e-4); PSUM accumulation stays fp32.
USE_F16_Z = True
# fp16 embeddings: Wcat/x^T operands and h tiles in fp16; S_hat PSUM
# accumulation stays f32. Halves the h SBUF/DMA and speeds PE.
USE_F16_EMB = True
# fp16 M^T matrices (values are small exact ints) and the rs3/rt3p
# operands feeding them: halves the dominant input DMA and speeds PE.
USE_F16_M = True
ZDT = F16 if USE_F16_Z else F32
EDT = F16 if USE_F16_EMB else F32
MDT = F16 if USE_F16_M else F32


def build_nc(trace_scopes=False):
    nc = bacc.Bacc(
        "TRN2", target_bir_lowering=False, debug=False, num_devices=NCORES)

    t_xsT = nc.dram_tensor("xsT", [CIN, SHARD], EDT, kind="ExternalInput")
    t_xtT = nc.dram_tensor("xtT", [CIN, N], EDT, kind="ExternalInput")
    t_MsT = nc.dram_tensor("MsT_shard", [N, SHARD], MDT, kind="ExternalInput")
    t_MtT = nc.dram_tensor("MtT", [N, N], MDT, kind="ExternalInput")
    t_Wcat = nc.dram_tensor("Wcat", [CIN, 512], EDT, kind="ExternalInput")
    t_W3 = nc.dram_tensor("W3", [R, R], F32, kind="ExternalInput")
    t_Wm1 = nc.dram_tensor("Wm1", [R, R], F32, kind="ExternalInput")
    t_Wm1n = nc.dram_tensor("Wm1neg", [R, R], F32, kind="ExternalInput")
    t_b3 = nc.dram_tensor("b3_col", [R, 1], F32, kind="ExternalInput")
    t_bm1 = nc.dram_tensor("bm1_col", [R, 1], F32, kind="ExternalInput")
    t_rsT = nc.dram_tensor("rsT", [STEPS * R, N], F32, kind="ExternalInput")
    t_rsTsh = nc.dram_tensor(
        "rsT_shard", [STEPS * R, SHARD], F32, kind="ExternalInput")
    # 8 sub-masks: mask_sub[32b+k, m] = Wm2[k] iff m == 4*sub+b
    t_w2s = nc.dram_tensor("W2stack", [8 * 128, R], ZDT, kind="ExternalInput")
    # summask[32c+k, m] = (m == k): sums 4 stacked [32, N] partials
    t_smask = nc.dram_tensor("SumMask", [128, R], F32, kind="ExternalInput")

    t_S0 = nc.dram_tensor("S0_out", [SHARD, N], F32, kind="ExternalOutput")
    t_SL = nc.dram_tensor("SL_out", [SHARD, N], F32, kind="ExternalOutput")

    with tile.TileContext(nc) as tc, ExitStack() as ctx:
        sb = ctx.enter_context(tc.tile_pool(name="sb", bufs=1))
        sc = ctx.enter_context(tc.tile_pool(name="sc", bufs=1))
        zz = ctx.enter_context(tc.tile_pool(name="zz", bufs=6))
        ps = ctx.enter_context(tc.tile_pool(name="ps", bufs=2, space="PSUM"))
        psd = ctx.enter_context(tc.tile_pool(name="psd", bufs=1, space="PSUM"))
        dram = ctx.enter_context(tc.tile_pool(name="dram", bufs=1, space="DRAM"))

        for rr in range(REPEAT_ALL):
          # ---------------- constants & weights ----------------
          Wcat = sb.tile([CIN, 512], EDT, tag="Wcat")
          nc.sync.dma_start(Wcat[:], t_Wcat[:, :])
          W3 = sb.tile([R, R], F32, tag="W3")
          nc.sync.dma_start(W3[:], t_W3[:, :])
          Wm1 = sb.tile([R, R], F32, tag="Wm1")
          nc.sync.dma_start(Wm1[:], t_Wm1[:, :])
          Wm1n = sb.tile([R, R], F32, tag="Wm1n")
          nc.sync.dma_start(Wm1n[:], t_Wm1n[:, :])
          b3 = sb.tile([R, 1], F32, tag="b3")
          nc.sync.dma_start(b3[:], t_b3[:, :])
          bm1 = sb.tile([R, 1], F32, tag="bm1")
          nc.sync.dma_start(bm1[:], t_bm1[:, :])
          w2s = sb.tile([128, 8 * R], ZDT, tag="w2s")
          for sub in range(8):
              nc.sync.dma_start(
                  w2s[:, sub * R:(sub + 1) * R],
                  t_w2s[sub * 128:(sub + 1) * 128, :])
          smask = sb.tile([128, R], F32, tag="smask")
          nc.sync.dma_start(smask[:], t_smask[:, :])

          xsT = sb.tile([CIN, SHARD], EDT, tag="xsT")
          nc.sync.dma_start(xsT[:], t_xsT[:, :])
          xtT = sb.tile([CIN, N], EDT, tag="xtT")
          nc.sync.dma_start(xtT[:], t_xtT[:, :])

          # M^T blocks, column-blocked: block b at columns [b*N, (b+1)*N)
          MtT = sb.tile([128, NB * N], MDT, tag="MtT")
          for b in range(NB):
              nc.sync.dma_start(
                  MtT[:, b * N:(b + 1) * N], t_MtT[b * 128:(b + 1) * 128, :])
          MsT = sb.tile([128, NB * SHARD], MDT, tag="MsT")
          for b in range(NB):
              nc.sync.dma_start(
                  MsT[:, b * SHARD:(b + 1) * SHARD],
                  t_MsT[b * 128:(b + 1) * 128, :])

          rsT = sb.tile([R, STEPS * N], F32, tag="rsT")
          for s in range(STEPS):
              nc.sync.dma_start(
                  rsT[:, s * N:(s + 1) * N], t_rsT[s * R:(s + 1) * R, :])
          rsTsh = sb.tile([R, STEPS * SHARD], F32, tag="rsTsh")
          for s in range(STEPS):
              nc.sync.dma_start(
                  rsTsh[:, s * SHARD:(s + 1) * SHARD],
                  t_rsTsh[s * R:(s + 1) * R, :])

          # ---------------- embeddings h^T = Wcat^T @ x^T ----------------
          htT = sb.tile([128, 4 * N], EDT, tag="htT")   # cout-block co at cols [co*N, ...)
          hsT = sb.tile([128, 4 * SHARD], EDT, tag="hsT")
          for co in range(4):
              for jh in range(2):
                  ph = ps.tile([128, 512], F32, tag="mm")
                  nc.tensor.matmul(
                      ph[:], Wcat[:, co * 128:(co + 1) * 128],
                      xtT[:, jh * 512:(jh + 1) * 512])
                  nc.vector.tensor_copy(
                      htT[:, co * N + jh * 512:co * N + (jh + 1) * 512], ph[:])
              ph2 = ps.tile([128, 512], F32, tag="mm")
              nc.tensor.matmul(
                  ph2[:, 0:SHARD], Wcat[:, co * 128:(co + 1) * 128], xsT[:])
              nc.scalar.copy(
                  hsT[:, co * SHARD:(co + 1) * SHARD], ph2[:, 0:SHARD])

          # ---------------- S_hat = h_s @ h_t^T (shard rows) ----------------
          S_hat = sb.tile([SHARD, N], F32, tag="S_hat")
          for jh in range(2):
              pS = ps.tile([128, 512], F32, tag="mm")
              for co in range(4):
                  nc.tensor.matmul(
                      pS[:],
                      hsT[:, co * SHARD:(co + 1) * SHARD],
                      htT[:, co * N + jh * 512:co * N + (jh + 1) * 512],
                      start=(co == 0), stop=(co == 3))
              nc.vector.tensor_copy(S_hat[:, jh * 512:(jh + 1) * 512], pS[:])

          # ---------------- per-step precompute (A-side etc.) ----------------
          # rs3 = r_s @ W3, node-block b at cols [s*NB*R + b*R, ...)
          rs3 = sb.tile([128, STEPS * NB * R], MDT, tag="rs3")
          rs3sh = sb.tile([SHARD, STEPS * R], F32, tag="rs3sh")
          A4 = sb.tile([128, STEPS * G], F32, tag="A4")
          for s in range(STEPS):
              pr = ps.tile([128, NB * R], F32, tag="prt")
              for b in range(NB):
                  nc.tensor.matmul(
                      pr[:, b * R:(b + 1) * R],
                      rsT[:, s * N + b * 128:s * N + (b + 1) * 128], W3[:])
              nc.scalar.copy(
                  rs3[:, s * NB * R:(s + 1) * NB * R], pr[:])
              prs = ps.tile([128, 512], F32, tag="mm")
              nc.tensor.matmul(
                  prs[:, 0:R],
                  rsTsh[:, s * SHARD:(s + 1) * SHARD], W3[:])
              nc.scalar.copy(rs3sh[:, s * R:(s + 1) * R], prs[:, 0:R])

              # tmp_s^T [R, SHARD] = sum_b (rs3_b as lhsT) @ MsT_b
              pts = ps.tile([128, 512], F32, tag="mm")
              for b in range(NB):
                  nc.tensor.matmul(
                      pts[0:R, 0:SHARD],
                      rs3[:, (s * NB + b) * R:(s * NB + b + 1) * R],
                      MsT[:, b * SHARD:(b + 1) * SHARD],
                      start=(b == 0), stop=(b == NB - 1))
              osT = sc.tile([R, SHARD], F32, tag="osT")
              nc.scalar.activation(osT[:], pts[0:R, 0:SHARD], AF.Relu,
                                   bias=b3[:])
              pA = ps.tile([128, 512], F32, tag="mm")
              nc.tensor.matmul(pA[0:R, 0:SHARD], Wm1[:], osT[:])
              AT = sc.tile([R, SHARD], F32, tag="AT")
              nc.scalar.activation(AT[:], pA[0:R, 0:SHARD], AF.Identity,
                                   bias=bm1[:])
              # A4[32b+k, s*G+g] = AT[k, 4g+b]
              for b in range(4):
                  nc.sync.dma_start(
                      A4[32 * b:32 * (b + 1), s * G:(s + 1) * G],
                      AT[:, b::4])
          # ---------------- consensus steps ----------------
          if REPEAT > 1:
              S_hat0 = sb.tile([SHARD, N], F32, tag="S_hat0")
              nc.vector.tensor_copy(S_hat0[:], S_hat[:])
          for rep in range(REPEAT):
            if rep > 0:
                nc.vector.tensor_copy(S_hat[:], S_hat0[:])
            for s in range(STEPS):
              scope = tc.named_scope(f"step{s}") if trace_scopes else None
              if scope is not None:
                  scope.__enter__()
              # softmax over rows of S_hat
              nmax = sc.tile([SHARD, 1], F32, tag="nmax")
              nc.vector.tensor_reduce(
                  nmax[:], S_hat[:, :], axis=mybir.AxisListType.X,
                  op=OP.max, negate=True)
              E = sc.tile([SHARD, N], F32, tag="E")
              rsum = sc.tile([SHARD, 1], F32, tag="rsum")
              nc.scalar.activation(
                  E[:], S_hat[:, :], AF.Exp, bias=nmax[:], accum_out=rsum[:])
              rinv = sc.tile([SHARD, 1], F32, tag="rinv")
              nc.vector.reciprocal(rinv[:], rsum[:])
              if s == 0:
                  Snorm = sc.tile([SHARD, N], F32, tag="Snorm")
                  nc.vector.tensor_scalar_mul(Snorm[:], E[:], rinv[:])
                  nc.sync.dma_start(t_S0[:, :], Snorm[:])

              # r_t3 partials: lhsT = E j-blocks, rhs = rinv-scaled rs3 shard
              rsc = sc.tile([SHARD, R], F32, tag="rsc")
              nc.vector.tensor_scalar_mul(
                  rsc[:], rs3sh[:, s * R:(s + 1) * R], rinv[:])
              rt3p = sc.tile([128, NB * R], MDT, tag="rt3p")
              prt = ps.tile([128, NB * R], F32, tag="prt")
              for jb in range(NB):
                  nc.tensor.matmul(
                      prt[:, jb * R:(jb + 1) * R],
                      E[:, jb * 128:(jb + 1) * 128], rsc[:])
              nc.scalar.copy(rt3p[:], prt[:])

              # tmp_t^T partial [R, N] = sum_b rt3p_b @ MtT_b
              ptt = psd.tile([R, N], F32, tag="ptt")
              for jh in range(2):
                  for b in range(NB):
                      nc.tensor.matmul(
                          ptt[:, jh * 512:(jh + 1) * 512],
                          rt3p[:, b * R:(b + 1) * R],
                          MtT[:, b * N + jh * 512:b * N + (jh + 1) * 512],
                          start=(b == 0), stop=(b == NB - 1))
              ttp = sc.tile([R, N], F32, tag="ttp")
              nc.scalar.copy(ttp[:], ptt[:])

              ar_in = dram.tile([R, N], F32, tag=f"ar_in{rr}_{s}")
              ag_out = dram.tile([NCORES * R, N], F32, tag=f"ar_out{rr}_{s}")
              nc.sync.dma_start(ar_in[:], ttp[:])
              nc.gpsimd.collective_compute(
                  "AllGather", OP.bypass,
                  replica_groups=[list(range(NCORES))],
                  ins=[ar_in[:].opt()], outs=[ag_out[:].opt()])
              # gathered partials: rank c at rows [32c, 32c+32).
              # Load as two [128, N] tiles (4 ranks each) and sum the
              # ranks with two accumulating mask matmuls per j-half.
              agt = sc.tile([128, 2 * N], F32, tag="agt")
              for h in range(2):
                  nc.sync.dma_start(
                      agt[:, h * N:(h + 1) * N],
                      ag_out[h * 128:(h + 1) * 128, :])
              ptt2 = psd.tile([R, N], F32, tag="ptt")
              for jh in range(2):
                  for h in range(2):
                      nc.tensor.matmul(
                          ptt2[:, jh * 512:(jh + 1) * 512],
                          smask[:],
                          agt[:, h * N + jh * 512:h * N + (jh + 1) * 512],
                          start=(h == 0), stop=(h == 1),
                          skip_group_check=True)

              # o_t^T = relu(tmp_t^T + b3);  Brep rows 0-31 = -(Wm1^T o_t^T)
              otT = sc.tile([R, N], F32, tag="otT")
              nc.scalar.activation(otT[:], ptt2[:], AF.Relu, bias=b3[:])
              pB = psd.tile([R, N], F32, tag="ptt")
              for jh in range(2):
                  nc.tensor.matmul(
                      pB[:, jh * 512:(jh + 1) * 512], Wm1n[:],
                      otT[:, jh * 512:(jh + 1) * 512])
              Brep = sc.tile([128, N], ZDT, tag="Brep")
              nc.scalar.copy(Brep[0:R, :], pB[:])
              for b in range(1, 4):
                  nc.sync.dma_start(Brep[R * b:R * (b + 1), :], Brep[0:R, :])

              # delta: z = relu(A4[:,g] - B) then Wm2-contract over channels.
              # Group g covers i-rows [4g, 4g+4); super-group g' = g//8 is a
              # 32-partition PSUM stripe accumulated over sub = g%8 via a
              # [128, 32] w2 mask with nonzeros in columns 4*sub..4*sub+3.
              # Iterate sub-major so consecutive matmuls hit different
              # col-group strips (concurrent in the PE array).
              dpsum = psd.tile([128, N], F32, tag="dpsum")
              order = [gp * 8 + su for su in range(8) for gp in range(4)]
              for gi, g in enumerate(order):
                  z = zz.tile([128, N], ZDT, tag="z")
                  if gi % 3 == 2:
                      # ACT computes the same relu(A - B): in=Brep holds -B
                      nc.scalar.activation(
                          z[:], Brep[:], AF.Relu,
                          bias=A4[:, s * G + g:s * G + g + 1])
                  else:
                      nc.vector.tensor_scalar(
                          z[:], Brep[:],
                          A4[:, s * G + g:s * G + g + 1], 0.0,
                          op0=OP.add, op1=OP.max)
                  for jh in range(2):
                      sub, gp = g % 8, g // 8
                      nc.tensor.matmul(
                          dpsum[32 * gp:32 * (gp + 1),
                                jh * 512:(jh + 1) * 512],
                          w2s[:, sub * R:(sub + 1) * R],
                          z[:, jh * 512:(jh + 1) * 512],
                          start=(sub == 0), stop=(sub == 7),
                          skip_group_check=True,
                          tile_position=(0, 32 * gp))
              for jh in range(2):
                  nc.vector.tensor_tensor(
                      out=S_hat[:, jh * 512:(jh + 1) * 512],
                      in0=S_hat[:, jh * 512:(jh + 1) * 512],
                      in1=dpsum[:, jh * 512:(jh + 1) * 512],
                      op=OP.add)
              if scope is not None:
                  scope.__exit__(None, None, None)

          # ---------------- final softmax ----------------
          nmax = sc.tile([SHARD, 1], F32, tag="nmax")
          nc.vector.tensor_reduce(
              nmax[:], S_hat[:, :], axis=mybir.AxisListType.X,
              op=OP.max, negate=True)
          E = sc.tile([SHARD, N], F32, tag="E")
          rsum = sc.tile([SHARD, 1], F32, tag="rsum")
          nc.scalar.activation(
              E[:], S_hat[:, :], AF.Exp, bias=nmax[:], accum_out=rsum[:])
          rinv = sc.tile([SHARD, 1], F32, tag="rinv")
          nc.vector.reciprocal(rinv[:], rsum[:])
          SL = sc.tile([SHARD, N], F32, tag="Snorm")
          nc.vector.tensor_scalar_mul(SL[:], E[:], rinv[:])
          nc.sync.dma_start(t_SL[:, :], SL[:])

    nc.compile()
    return nc


def _host_prep(inputs, index_n1, index_n2, edge_index_s, edge_index_t,
               W1, W2, W3, b3, Wm1, bm1, Wm2, bm2, rs_all):
    """Per-core input maps (numpy only: index/layout preprocessing)."""
    f32 = np.float32
    edt = np.float16 if USE_F16_EMB else f32
    mdt = np.float16 if USE_F16_M else f32
    x = np.asarray(inputs, f32)
    idx_s = np.asarray(index_n1).astype(np.int64)
    idx_t = np.asarray(index_n2).astype(np.int64)
    xsT_full = np.ascontiguousarray(x[idx_s].T.astype(edt))   # [CIN, N]
    xtT = np.ascontiguousarray(x[idx_t].T.astype(edt))        # [CIN, N]

    def mT(edge_index):
        src = np.asarray(edge_index[0]).astype(np.int64)
        dst = np.asarray(edge_index[1]).astype(np.int64)
        M = np.zeros((N, N), f32)          # M^T[src, dst] = (I+Adj)^T
        np.add.at(M, (src, dst), 1.0)
        M[np.arange(N), np.arange(N)] += 1.0
        return M

    MsT = mT(edge_index_s).astype(mdt)
    MtT = np.ascontiguousarray(mT(edge_index_t).astype(mdt))
    Wcat = np.ascontiguousarray(
        np.concatenate([np.asarray(W1, f32), np.asarray(W2, f32)],
                       axis=1).astype(edt))
    W3a = np.ascontiguousarray(np.asarray(W3, f32))
    Wm1a = np.ascontiguousarray(np.asarray(Wm1, f32))
    b3c = np.ascontiguousarray(np.asarray(b3, f32).reshape(R, 1))
    bm1c = np.ascontiguousarray(np.asarray(bm1, f32).reshape(R, 1))
    w2 = np.asarray(Wm2, f32).reshape(R)
    rs = np.asarray(rs_all, f32)
    rsT = np.ascontiguousarray(
        np.transpose(rs, (0, 2, 1)).reshape(STEPS * R, N))

    zdt = np.float16 if USE_F16_Z else f32
    w2s = np.zeros((8 * 128, R), zdt)
    for sub in range(8):
        for b in range(4):
            w2s[sub * 128 + 32 * b:sub * 128 + 32 * (b + 1),
                4 * sub + b] = w2
    smask = np.zeros((128, R), f32)
    for c in range(4):
        smask[32 * c:32 * (c + 1), :] = np.eye(R, dtype=f32)

    in_maps = []
    for c in range(NCORES):
        sl = slice(c * SHARD, (c + 1) * SHARD)
        m = {
            "xsT": np.ascontiguousarray(xsT_full[:, sl]),
            "xtT": xtT,
            "MsT_shard": np.ascontiguousarray(MsT[:, sl]),
            "MtT": MtT,
            "Wcat": Wcat,
            "W3": W3a,
            "Wm1": Wm1a,
            "Wm1neg": np.ascontiguousarray(-Wm1a),
            "b3_col": b3c,
            "bm1_col": bm1c,
            "rsT": rsT,
            "rsT_shard": np.ascontiguousarray(
                np.transpose(rs[:, sl, :], (0, 2, 1)).reshape(
                    STEPS * R, SHARD)),
            "W2stack": w2s,
            "SumMask": smask,
        }
        in_maps.append(m)
    return in_maps


_NC_CACHE = None


def kernel(**inputs):
    global _NC_CACHE
    in_maps = _host_prep(**inputs)
    if _NC_CACHE is None:
        _NC_CACHE = build_nc()
    res = run_bass_kernel_spmd(
        _NC_CACHE, in_maps, core_ids=list(range(NCORES)))
    S0 = np.concatenate([r["S0_out"] for r in res.results], axis=0)
    SL = np.concatenate([r["SL_out"] for r in res.results], axis=0)
    return S0, SL
